# revision 1
# baseline (speedup 1.0000x reference)
"""Chamfer distance kernel for Trainium2 (8 NeuronCores, bass/tile).

Problem: X [8, 8192, 3], Y [8, 8192, 3] fp32.
  out[b] = mean_n min_m ||x_n - y_m||^2 + mean_m min_n ||x_n - y_m||^2

Strategy:
  - Data parallel over batch: core b handles batch b.
  - Distance matrix W[n,m] = |x_n|^2 + |y_m|^2 - 2 x.y is produced directly by
    the PE array as a single K=24 matmul per tile: the contraction dimension
    carries an error-free triple-bf16 splitting of X, -2Y, |x|^2, |y|^2 and
    ones, so PSUM tiles hold fp32-accurate distances at bf16 streaming speed
    (1 cycle/row vs 4 for native fp32 matmul).
  - ScalarE (ACT) casts each PSUM tile to fp16 in SBUF (the only engine with
    spare elementwise throughput; it cannot do min).
  - VectorE (DVE) does both min paths in fp16 at 2x_1P rate:
      row path: pairwise-min fold of the four 2048-wide supertiles of each
                n-tile, then a free-axis min-reduce -> rminv[:, i]
      col path: running elementwise min into a persistent [128, 8192]
                accumulator.
  - Column mins need a partition-axis reduce: PE-transpose 128x128 chunks
    (after an ACT cast back to fp32) and free-axis min-reduce each.
  - Host: means of the returned 2*8192 mins per batch.
"""

import os
import sys

sys.path.insert(0, "/opt/trn_rl_repo")

import numpy as np

B, N, M, D = 8, 8192, 8192, 3
KROWS = 24
SUPER = 2048  # psum supertile free size (4 banks)
FILL = 30000.0  # > any squared distance (~80), well below fp16 max

_CACHE = {}


def _split3_bf16(v):
    """Error-free-ish triple bf16 split: v ~= s0+s1+s2 to ~26 mantissa bits."""
    import ml_dtypes

    bf = ml_dtypes.bfloat16
    v = v.astype(np.float64)
    s0 = v.astype(bf)
    r1 = v - s0.astype(np.float64)
    s1 = r1.astype(bf)
    r2 = r1 - s1.astype(np.float64)
    s2 = r2.astype(bf)
    return s0, s1, s2


def _augment(X, Y):
    """Build [B, 24, N] bf16 lhsT rows and [B, 24, M] rhs rows such that
    sum_k XAT[k,n] * YAT[k,m] = |x_n|^2 + |y_m|^2 - 2 x_n.y_m  (fp32-accurate).
    """
    import ml_dtypes

    bf = ml_dtypes.bfloat16
    Xf = np.asarray(X, np.float64)
    Yf = np.asarray(Y, np.float64)
    X2 = (Xf * Xf).sum(-1)  # [B, N]
    Y2 = (Yf * Yf).sum(-1)  # [B, M]
    xs = _split3_bf16(np.moveaxis(Xf, -1, 1))  # 3 x [B, D, N]
    ys = _split3_bf16(np.moveaxis(-2.0 * Yf, -1, 1))  # 3 x [B, D, M]
    a = _split3_bf16(X2)  # 3 x [B, N]
    b = _split3_bf16(Y2)  # 3 x [B, M]

    nb, mb = X.shape[1], Y.shape[1]
    XAT = np.zeros((B, KROWS, nb), bf)
    YAT = np.zeros((B, KROWS, mb), bf)
    # cross terms: pairings (i,j) with i+j <= 2 capture products to ~2^-26
    pairs = [(0, 0), (0, 1), (1, 0), (0, 2), (1, 1), (2, 0)]
    r = 0
    for d in range(D):
        for (i, j) in pairs:
            XAT[:, r, :] = xs[i][:, d, :]
            YAT[:, r, :] = ys[j][:, d, :]
            r += 1
    for i in range(3):  # |x|^2 splits vs ones
        XAT[:, r, :] = a[i]
        YAT[:, r, :] = np.ones((B, mb), bf)
        r += 1
    for i in range(3):  # ones vs |y|^2 splits
        XAT[:, r, :] = np.ones((B, nb), bf)
        YAT[:, r, :] = b[i]
        r += 1
    assert r == KROWS
    return XAT, YAT


_CDVE = {}


def _register_minmin_dveop():
    """Register a custom DVE op: out = min(in0,in1); accum = min(s0, min(out)).

    Same semantics as InstTensorTensorReduce (which faults at runtime on this
    toolchain) but through the ant custom-DVE uop table, which production
    accum ops (TENSOR_MASK_REDUCE etc.) use successfully.
    """
    if "op" in _CDVE:
        return _CDVE["op"]
    import numpy as np
    from concourse import dve_ops
    from concourse.dve_spec import Spec, Src0, Src1, minn, lower, _has_src1
    from concourse.dve_uop import DveOpSpec

    def _ref(in0, in1, s0, s1, imm2):
        b = np.minimum(in0.astype(np.float32), in1.astype(np.float32))
        seed = np.asarray(s0, np.float32).reshape(-1, 1)
        acc = np.minimum(b.reshape(b.shape[0], -1).min(axis=-1, keepdims=True), seed)
        return b, acc

    spec = Spec(body=minn(Src0, Src1), accum=minn, accum_init=dve_ops.C0,
                reference=_ref)
    op = dve_ops.DveOp("CHAMFER_MINMIN_ANT", spec, subdim=False, uops_sha={},
                       perf_en={"v3": True, "v4": True})
    # pin shas dynamically (computed == pinned by construction)
    row = max(dve_ops._SUB_OPCODE_FOR_NAME.values()) + 1
    assert row < 0x20
    dve_ops._SUB_OPCODE_FOR_NAME[op.name] = row
    for ver in ("v3", "v4"):
        try:
            s = DveOpSpec(name=op.name, opcode=row, uops=lower(spec, ver=ver),
                          rd1_en=_has_src1(spec))
            op.uops_sha[ver] = s.sha(ver)
        except Exception:
            pass
    dve_ops.OPS.append(op)
    dve_ops.CUSTOM_DVE_SPECS[op.name] = spec
    _CDVE["op"] = op
    return op


def build_module(n_rows=N, m_cols=M, repeat=1, gp_slices=(), mode="full",
                 half_dt="bfloat16", rowgroups=1, use_ttr=False, dma_cols=0,
                 use_cdve=True):
    """Build + compile the per-core bass program. Same program on all cores.

    repeat: run the main loop `repeat` times (idempotent mins) — used to
            measure device time as a wall-clock delta between repeat counts.
    gp_slices: unused (GPSIMD tensor_tensor is not legal on TRN2).
    mode: 'full' | 'mm' (matmuls + tiny cast probe) | 'mm_act' (no DVE min
          work) — engine-isolation probes for HW timing.
    half_dt: 'float16' or 'bfloat16' reduction dtype.
    """
    import concourse.bacc as bacc
    import concourse.mybir as mybir
    import concourse.tile as tile
    from concourse._compat import get_trn_type

    dt = mybir.dt
    hdt = getattr(dt, half_dt)
    op_min = mybir.AluOpType.min
    ax_x = mybir.AxisListType.X

    NT = n_rows // 128
    ST = m_cols // SUPER
    CT = m_cols // 128  # 128-column chunks for the transpose phase

    nc = bacc.Bacc(get_trn_type() or "TRN2", target_bir_lowering=False, debug=False)
    xat = nc.dram_tensor("xat", [KROWS, n_rows], dt.bfloat16, kind="ExternalInput")
    yat = nc.dram_tensor("yat", [KROWS, m_cols], dt.bfloat16, kind="ExternalInput")
    ident = nc.dram_tensor("ident", [128, 128], dt.float32, kind="ExternalInput")
    out = nc.dram_tensor("out", [128, NT + CT], dt.float32, kind="ExternalOutput")

    with tile.TileContext(nc) as tc:
        with (
            tc.tile_pool(name="const", bufs=1) as cpool,
            tc.tile_pool(name="acc", bufs=1) as apool,
            tc.tile_pool(name="res", bufs=1) as rpool,
        ):
            ident_sb = cpool.tile([128, 128], dt.float32)
            nc.sync.dma_start(ident_sb[:], ident[:])
            if rowgroups > 1:
                # replicate operands at partition offsets 0/32/64/96 so
                # matmuls can rotate PE row groups (LDWEIGHTS of group g
                # overlaps the in-flight MATMUL of group g-1)
                xat_sb = cpool.tile([128, n_rows], dt.bfloat16)
                yat_sb = cpool.tile([128, m_cols], dt.bfloat16)
                for g in range(rowgroups):
                    nc.sync.dma_start(xat_sb[32 * g : 32 * g + KROWS, :], xat[:])
                    nc.sync.dma_start(yat_sb[32 * g : 32 * g + KROWS, :], yat[:])
            else:
                xat_sb = cpool.tile([KROWS, n_rows], dt.bfloat16)
                yat_sb = cpool.tile([KROWS, m_cols], dt.bfloat16)
                nc.sync.dma_start(xat_sb[:], xat[:])
                nc.sync.dma_start(yat_sb[:], yat[:])

            cacc = apool.tile([128, m_cols], hdt)
            rminv = rpool.tile([128, NT], dt.float32)
            cminv = rpool.tile([128, CT], dt.float32)
            nc.vector.memset(cacc[:], FILL)
            if mode != "full":
                nc.vector.memset(rminv[:], 0.0)

            from contextlib import ExitStack

            GRAIN = 1024  # psum sub-tile (2 matmuls, 2 banks); 4 bufs = 8 banks
            NSUB = m_cols // GRAIN
            with (
                tc.tile_pool(name="w", bufs=3) as wpool,
                tc.tile_pool(name="rf", bufs=2) as rfpool,
                tc.tile_pool(name="ps", bufs=4, space="PSUM") as pspool,
            ):
                with ExitStack() as rep_ctx:
                    if repeat > 1:
                        # hardware loop: identical static body each iteration
                        # (mins are idempotent), used for timing measurements
                        rep_ctx.enter_context(tc.For_i(0, repeat, 1))
                    for i in range(NT):
                        # one contiguous fp16 W stripe per n-tile
                        wb = wpool.tile([128, m_cols], hdt, tag="w")
                        for sub in range(NSUB):
                            ps = pspool.tile([128, GRAIN], dt.float32)
                            for q in range(GRAIN // 512):
                                mo = sub * GRAIN + q * 512
                                if rowgroups > 1:
                                    g = (sub * (GRAIN // 512) + q) % rowgroups
                                    nc.tensor.matmul(
                                        ps[:, q * 512 : (q + 1) * 512],
                                        xat_sb[
                                            32 * g : 32 * g + KROWS,
                                            i * 128 : (i + 1) * 128,
                                        ],
                                        yat_sb[32 * g : 32 * g + KROWS, mo : mo + 512],
                                        start=True,
                                        stop=True,
                                        tile_position=(32 * g, 0),
                                    )
                                else:
                                    nc.tensor.matmul(
                                        ps[:, q * 512 : (q + 1) * 512],
                                        xat_sb[:, i * 128 : (i + 1) * 128],
                                        yat_sb[:, mo : mo + 512],
                                        start=True,
                                        stop=True,
                                    )
                            if mode == "mm":
                                # probe: consume each psum bank cheaply so no
                                # matmul is dead-code eliminated
                                for q in range(GRAIN // 512):
                                    nc.scalar.copy(
                                        wb[:, sub * 64 + q * 16 : sub * 64 + q * 16 + 16],
                                        ps[:, q * 512 : q * 512 + 16],
                                    )
                                continue
                            nc.scalar.copy(
                                wb[:, sub * GRAIN : (sub + 1) * GRAIN], ps[:]
                            )
                        if mode == "mm":
                            continue
                        if mode == "mm_act":
                            # probe: tiny DVE consumer, no real min work
                            nc.vector.tensor_tensor(
                                cacc[:, :64], cacc[:, :64], wb[:, :64], op_min
                            )
                            continue
                        # col path: running min into the persistent accumulator.
                        # The trailing dma_cols columns go through the SDMA CCE
                        # (SWDGE dma accum) to offload VectorE.
                        dvw = m_cols - dma_cols
                        CW = 4096  # fewer, larger DVE ops
                        off = 0
                        while off < dvw:
                            cw = min(CW, dvw - off)
                            nc.vector.tensor_tensor(
                                cacc[:, off : off + cw],
                                cacc[:, off : off + cw],
                                wb[:, off : off + cw],
                                op_min,
                            )
                            off += cw
                        if dma_cols:
                            nc.gpsimd.dma_start(
                                out=cacc[:, dvw:m_cols],
                                in_=wb[:, dvw:m_cols],
                                accum_op=op_min,
                            )
                        # row path: fold the stripe in half repeatedly, then reduce
                        half = m_cols // 2
                        f = rfpool.tile([128, half], hdt, tag="rf")
                        if use_cdve:
                            # one dual-output custom-DVE op: exact row min
                            nc.vector._custom_dve(
                                _register_minmin_dveop(),
                                out=f[:],
                                in0=wb[:, :half],
                                in1=wb[:, half:],
                                s0=float(FILL),
                                accum_out=rminv[:, i : i + 1],
                            )
                            continue
                        if use_ttr:
                            # single dual-output op: f = min(lo, hi) and
                            # accum_out = min(FILL, min_free(f)) = exact rowmin
                            nc.vector.tensor_tensor_reduce(
                                out=f[:],
                                in0=wb[:, :half],
                                in1=wb[:, half:],
                                scale=1.0,
                                scalar=float(FILL),
                                op0=op_min,
                                op1=op_min,
                                accum_out=rminv[:, i : i + 1],
                            )
                            continue
                        nc.vector.tensor_tensor(
                            f[:], wb[:, :half], wb[:, half:], op_min
                        )
                        width = half
                        while width > 128:
                            h = width // 2
                            nc.vector.tensor_tensor(
                                f[:, 0:h], f[:, 0:h], f[:, h:width], op_min
                            )
                            width = h
                        nc.vector.tensor_reduce(
                            rminv[:, i : i + 1], f[:, 0:width], axis=ax_x, op=op_min
                        )

            # col path finalization: partition-axis min via PE transpose.
            # 4 transposed 128x128 chunks share one PSUM bank tile; a single
            # 3D-AP reduce then emits 4 column-min entries at once.
            with (
                tc.tile_pool(name="c32", bufs=2) as c32pool,
                tc.tile_pool(name="pst", bufs=4, space="PSUM") as ptpool,
            ):
                for g in range(m_cols // SUPER):
                    c32 = c32pool.tile([128, SUPER], dt.float32)
                    nc.scalar.copy(c32[:], cacc[:, g * SUPER : (g + 1) * SUPER])
                    for c4 in range(SUPER // 512):
                        pt = ptpool.tile([128, 4, 128], dt.float32)
                        for c in range(4):
                            nc.tensor.transpose(
                                pt[:, c, :],
                                c32[:, (c4 * 4 + c) * 128 : (c4 * 4 + c + 1) * 128],
                                ident_sb[:],
                            )
                        ci = g * (SUPER // 128) + c4 * 4
                        nc.vector.tensor_reduce(
                            cminv[:, ci : ci + 4], pt[:], axis=ax_x, op=op_min
                        )

            nc.sync.dma_start(out[:, :NT], rminv[:])
            nc.sync.dma_start(out[:, NT:], cminv[:])

    nc.compile()
    return nc


def _get_module():
    rep = int(os.environ.get("CHAMFER_REPEAT", "1"))
    half = os.environ.get("CHAMFER_HALF", "bfloat16")
    rg = int(os.environ.get("CHAMFER_RG", "1"))
    key = ("nc", rep, half, rg)
    if key not in _CACHE:
        _CACHE[key] = build_module(repeat=rep, half_dt=half, rowgroups=rg)
    return _CACHE[key]


def kernel(X, Y):
    from concourse import bass_utils

    X = np.asarray(X)
    Y = np.asarray(Y)
    assert X.shape == (B, N, D) and Y.shape == (B, M, D)

    XAT, YAT = _augment(X, Y)
    ident = np.eye(128, dtype=np.float32)

    nc = _get_module()
    in_maps = [
        {"xat": XAT[b], "yat": YAT[b], "ident": ident} for b in range(B)
    ]
    trace = bool(int(os.environ.get("CHAMFER_TRACE", "0")))
    r = bass_utils.run_bass_kernel_spmd(
        nc, in_maps, core_ids=list(range(B)), trace=trace
    )
    _CACHE["last_results"] = r

    NT = N // 128
    outv = np.empty((B,), np.float32)
    for b in range(B):
        o = r.results[b]["out"]  # [128, NT + CT] fp32
        rmin = o[:, :NT].astype(np.float64)
        cmin = o[:, NT:].astype(np.float64)
        outv[b] = np.float32(rmin.mean() + cmin.mean())
    return outv



# revision 2
# speedup vs baseline: 1.3784x; 1.3784x over previous
"""Chamfer distance kernel v2 for Trainium2 (8 NeuronCores, bass/tile).

Problem: X [8, 8192, 3], Y [8, 8192, 3] fp32.
  out[b] = mean_n min_m ||x_n - y_m||^2 + mean_m min_n ||x_n - y_m||^2

Design (one batch per core):
  - W tiles [128, 8192] produced by the PE as a K=24 bf16 triple-split
    matmul (fp32-accurate distances). 16x 512-col matmuls per tile, with
    4-way tile_position row-group rotation (measured 3.5x faster than a
    single row group: 2.85us vs 9.96us per tile).
  - One custom DVE op (CHAMFER_F2X_ANT) consumes W:
      out = min(in0, in1)            (running column-min update)
      accum_out = min(s0, min(in1))  (row-min of the W stream only; the
        lowered accumulator is patched to tap Src1's carry lane so cacc
        values cannot contaminate row-mins)
    It carries a hand-built 2X_1PORT uop program (element 1 via
    SRC_*_HI lanes, pair-min at blk2 feeding the accumulator), so on
    bf16 operands it runs at 2 elem/lane/cycle; on fp32/PSUM operands
    the engine falls back to REGULAR.
  - Columns split: A-slice (CA) consumed fp32 straight from PSUM
    (REGULAR); B-slice (M-CA) cast by ACT to bf16 wb, then consumed at
    2x. This balances DVE against ACT.
  - Row means: 2 accum entries per tile (A + B slices) -> host combines.
  - Col means: PE-transpose of cacc chunks, DVE tensor_reduce -> cminv.
"""

import os
import sys

sys.path.insert(0, "/opt/trn_rl_repo")

import numpy as np

B, N, M, D = 8, 8192, 8192, 3
KROWS = 24
FILL = 30000.0

GRAIN = 2048
CA = int(os.environ.get("CHAMFER_CA", "2048"))
CB = M - CA
ROWG = int(os.environ.get("CHAMFER_RG", "4"))  # PE row-group rotation
NO2X = bool(int(os.environ.get("CHAMFER_NO2X", "0")))  # disable 2x (diagnostics)

_CACHE = {}


def _split3_bf16(v):
    import ml_dtypes

    bfdt = ml_dtypes.bfloat16
    v = v.astype(np.float64)
    s0 = v.astype(bfdt)
    r1 = v - s0.astype(np.float64)
    s1 = r1.astype(bfdt)
    r2 = r1 - s1.astype(np.float64)
    s2 = r2.astype(bfdt)
    return s0, s1, s2


def _augment(X, Y):
    """[B,24,N] lhsT rows and [B,24,M] rhs rows: sum_k XAT[k,n]*YAT[k,m] =
    |x_n|^2 + |y_m|^2 - 2 x_n.y_m to ~2^-26."""
    import ml_dtypes

    bfdt = ml_dtypes.bfloat16
    Xf = np.asarray(X, np.float64)
    Yf = np.asarray(Y, np.float64)
    X2 = (Xf * Xf).sum(-1)
    Y2 = (Yf * Yf).sum(-1)
    xs = _split3_bf16(np.moveaxis(Xf, -1, 1))
    ys = _split3_bf16(np.moveaxis(-2.0 * Yf, -1, 1))
    a = _split3_bf16(X2)
    b = _split3_bf16(Y2)

    nb, mb = X.shape[1], Y.shape[1]
    XAT = np.zeros((B, KROWS, nb), bfdt)
    YAT = np.zeros((B, KROWS, mb), bfdt)
    pairs = [(0, 0), (0, 1), (1, 0), (0, 2), (1, 1), (2, 0)]
    r = 0
    for d in range(D):
        for (i, j) in pairs:
            XAT[:, r, :] = xs[i][:, d, :]
            YAT[:, r, :] = ys[j][:, d, :]
            r += 1
    for i in range(3):
        XAT[:, r, :] = a[i]
        YAT[:, r, :] = np.ones((B, mb), bfdt)
        r += 1
    for i in range(3):
        XAT[:, r, :] = np.ones((B, nb), bfdt)
        YAT[:, r, :] = b[i]
        r += 1
    assert r == KROWS
    return XAT, YAT


def _register_fused(name, with_2x):
    """Fused min/min custom DVE op; optionally with a 2X_1PORT program.

    The 2x program requires all-bf16 operands including accum_out (fp32
    accum_out corrupts the accumulator on HW -- probed)."""
    from concourse import dve_ops
    from concourse.dve_spec import Spec, Src0, Src1, minn, lower, _has_src1
    from concourse.dve_uop import (
        DveOpSpec, AluInp, AluOp, UopConfig, InpSel, OutSel,
        OutPath, Trigger, DelayInp, ENABLE,
    )

    if name in _CACHE:
        return _CACHE[name]

    def _ref(in0, in1, s0, s1, imm2):
        b = np.minimum(in0.astype(np.float32), in1.astype(np.float32))
        seed = np.asarray(s0, np.float32).reshape(-1, 1)
        acc = np.minimum(
            in1.astype(np.float32).reshape(in1.shape[0], -1).min(-1, keepdims=True),
            seed,
        )
        return b, acc

    spec = Spec(body=minn(Src0, Src1), accum=minn, accum_init=dve_ops.C0,
                reference=_ref)
    op = dve_ops.DveOp(name, spec, subdim=False, uops_sha={}, perf_en={})
    row = max(dve_ops._SUB_OPCODE_FOR_NAME.values()) + 1
    assert row < 0x20
    dve_ops._SUB_OPCODE_FOR_NAME[op.name] = row
    dve_ops.OPS.append(op)
    dve_ops.CUSTOM_DVE_SPECS[op.name] = spec

    MIN, BYP = AluOp.MIN, AluOp.BYPASS
    PREV, CURR = AluInp.PREV_ALU_OUT, AluInp.CURR_ALU_OUT
    D_ = [AluInp.PREV_DELAY_0, AluInp.PREV_DELAY_1, AluInp.PREV_DELAY_2,
          AluInp.PREV_DELAY_3, AluInp.PREV_DELAY_4]

    def mk2x(seed):
        u = UopConfig()
        for j, sel in [(1, InpSel.SRC_0), (2, InpSel.SRC_1),
                       (3, InpSel.SRC_0_HI), (4, InpSel.SRC_1_HI),
                       (5, InpSel.CONST_0)]:
            u.enable_input(sel, j)
        dp = u.datapath_config
        dp[0].enable_alu(MIN, D_[0], D_[1])
        dp[0].pass_through_delay(1, 2, 3, 4)
        dp[1].enable_alu(MIN, D_[2], D_[3])
        dp[1].pass_through_delay(1, 2, 3, 4)
        dp[1].enable_delay_from_src(DelayInp.PREV_ALU_OUT, 0)  # o_lo
        dp[2].enable_alu(MIN, D_[1], D_[3])
        dp[2].pass_through_delay(0, 1, 3, 4)
        dp[2].enable_delay_from_src(DelayInp.PREV_ALU_OUT, 2)  # o_hi
        if seed:
            dp[3].enable_alu(BYP, D_[4], D_[4])
        else:
            dp[3].enable_alu(MIN, CURR, PREV)
        dp[3].pass_through_delay(0, 2, 4)
        dp[3].alu_out_a_enable = ENABLE
        for k in (4, 5, 6, 7):
            dp[k].enable_alu(BYP, PREV, PREV)
            dp[k].pass_through_delay(0, 2, 4)
            dp[k].alu_out_a_enable = ENABLE
        u.accum_enabled = ENABLE
        if seed:
            u.trigger = (Trigger.COUNT, Trigger.NONE, Trigger.NONE)
            u.repeat_count = 1
            u.next_uop = (1, 0, 0)
        else:
            u.enable_output(OutSel.DELAY_0, OutPath.WR0_LO)
            u.enable_output(OutSel.DELAY_2, OutPath.WR0_HI)
            u.require_inp0 = 1
            u.require_inp1 = 1
            u.trigger = (Trigger.SRC_TENSOR_DONE, Trigger.NONE, Trigger.NONE)
            u.next_uop = (0, 0, 0)
        return u

    for ver in ("v3",):
        uops = lower(spec, ver=ver)
        steady = uops[-1]
        patched = False
        for blk in steady.datapath_config:
            if blk.alu_src0 == AluInp.CURR_ALU_OUT and blk.alu_out_a_enable:
                assert blk.alu_src1 == AluInp.PREV_ALU_OUT
                blk.alu_src1 = AluInp.PREV_DELAY_1  # accum taps Src1, not body
                patched = True
                break
        assert patched
        if with_2x:
            s = DveOpSpec(name=op.name, opcode=row, uops=uops,
                          uops_2x=[mk2x(True), mk2x(False)], perf_max=1,
                          rd1_en=_has_src1(spec))
        else:
            s = DveOpSpec(name=op.name, opcode=row, uops=uops,
                          rd1_en=_has_src1(spec))
        s.validate(ver)
        op.uops_sha[ver] = s.sha(ver)
        dve_ops._COMPILE_CACHE[(op.name, ver)] = s
    _CACHE[name] = op
    return op


def build_module(repeat=1):
    import concourse.bacc as bacc
    import concourse.mybir as mybir
    import concourse.tile as tile
    from concourse._compat import get_trn_type

    dt = mybir.dt
    op_min = mybir.AluOpType.min
    ax_x = mybir.AxisListType.X
    fusedA = _register_fused("CHAMFER_FUSED_ANT", with_2x=False)
    fusedB = _register_fused("CHAMFER_F2X_ANT", with_2x=True)

    NT = N // 128
    NG = M // GRAIN
    ga = CA // GRAIN
    assert CA % GRAIN == 0

    nc = bacc.Bacc(get_trn_type() or "TRN2", target_bir_lowering=False,
                   debug=False)
    xat = nc.dram_tensor("xat", [KROWS, N], dt.bfloat16, kind="ExternalInput")
    yat = nc.dram_tensor("yat", [KROWS, M], dt.bfloat16, kind="ExternalInput")
    ident = nc.dram_tensor("ident", [128, 128], dt.float32, kind="ExternalInput")
    out = nc.dram_tensor("out", [128, NT + M // 128], dt.float32,
                         kind="ExternalOutput")
    outb = nc.dram_tensor("outb", [128, NT], dt.bfloat16,
                          kind="ExternalOutput")

    def fused_op(op, out_, in0, in1, accum_out, twox):
        bi = nc.vector._custom_dve(op, out=out_, in0=in0, in1=in1,
                                   s0=float(FILL), accum_out=accum_out)
        if twox and not NO2X:
            bi.ins.perf_max = 1
        return bi

    with tile.TileContext(nc) as tc:
        with (
            tc.tile_pool(name="const", bufs=1) as cpool,
            tc.tile_pool(name="acc", bufs=1) as apool,
            tc.tile_pool(name="res", bufs=1) as rpool,
        ):
            ident_sb = cpool.tile([128, 128], dt.float32)
            identb = cpool.tile([128, 128], dt.bfloat16)
            nc.sync.dma_start(ident_sb[:], ident[:])
            nc.scalar.copy(identb[:], ident_sb[:])
            if ROWG > 1:
                # replicate operands at partition offsets 0/32/64/96 so
                # matmuls rotate PE row groups (overlapped weight loads)
                xat_sb = cpool.tile([128, N], dt.bfloat16)
                yat_sb = cpool.tile([128, M], dt.bfloat16)
                for g in range(ROWG):
                    nc.sync.dma_start(xat_sb[32 * g:32 * g + KROWS, :], xat[:])
                    nc.sync.dma_start(yat_sb[32 * g:32 * g + KROWS, :], yat[:])
            else:
                xat_sb = cpool.tile([KROWS, N], dt.bfloat16)
                yat_sb = cpool.tile([KROWS, M], dt.bfloat16)
                nc.sync.dma_start(xat_sb[:], xat[:])
                nc.sync.dma_start(yat_sb[:], yat[:])

            cacca = apool.tile([128, CA], dt.float32, name="cacca") if CA else None
            caccb = apool.tile([128, CB], dt.bfloat16, name="caccb") if CB else None
            rminva = rpool.tile([128, NT], dt.float32)
            rminvb = rpool.tile([128, NT], dt.bfloat16)
            cminv = rpool.tile([128, M // 128], dt.float32)
            if CA:
                nc.vector.memset(cacca[:], FILL)
            if CB:
                nc.vector.memset(caccb[:], FILL)
            nc.vector.memset(rminva[:], FILL)
            nc.vector.memset(rminvb[:], FILL)

            with (
                tc.tile_pool(name="wb", bufs=2) as wbpool,
                tc.tile_pool(name="ps", bufs=2, space="PSUM") as pspool,
            ):
                # Software-pipeline the B-slice consumption by one tile:
                # fusedB(i-1) is emitted after tile i's casts are queued, so
                # the in-order DVE never stalls waiting for the current
                # tile's ACT casts.
                pend = [None]

                def flush_pend():
                    prev = pend[0]
                    if prev is not None:
                        fused_op(fusedB, caccb[:], caccb[:], prev[0],
                                 rminvb[:, prev[1]:prev[1] + 1], twox=True)
                        pend[0] = None

                def emit_tile(i):
                    wb = wbpool.tile([128, CB], dt.bfloat16,
                                     name="wb", tag="w") if CB else None
                    for g in range(NG):
                        ps = pspool.tile([128, GRAIN], dt.float32, tag="g")
                        for q in range(GRAIN // 512):
                            mo = g * GRAIN + q * 512
                            if ROWG > 1:
                                rg = (g * (GRAIN // 512) + q) % ROWG
                                nc.tensor.matmul(
                                    ps[:, q * 512:(q + 1) * 512],
                                    xat_sb[32 * rg:32 * rg + KROWS,
                                           i * 128:(i + 1) * 128],
                                    yat_sb[32 * rg:32 * rg + KROWS,
                                           mo:mo + 512],
                                    start=True, stop=True,
                                    tile_position=(32 * rg, 0),
                                )
                            else:
                                nc.tensor.matmul(
                                    ps[:, q * 512:(q + 1) * 512],
                                    xat_sb[:, i * 128:(i + 1) * 128],
                                    yat_sb[:, mo:mo + 512],
                                    start=True, stop=True,
                                )
                        if g < ga:
                            o = g * GRAIN
                            fused_op(fusedA, cacca[:, o:o + GRAIN],
                                     cacca[:, o:o + GRAIN], ps[:],
                                     rminva[:, i:i + 1], twox=False)
                        else:
                            o = (g - ga) * GRAIN
                            nc.scalar.copy(wb[:, o:o + GRAIN], ps[:])
                    if CB:
                        flush_pend()
                        pend[0] = (wb, i)

                if repeat > 1:
                    with tc.For_i(0, repeat, 1):
                        for i in range(NT):
                            emit_tile(i)
                        flush_pend()
                else:
                    for i in range(NT):
                        emit_tile(i)
                    flush_pend()

            # col finalization: PE transpose 128-chunks, DVE reduce
            with tc.tile_pool(name="pst", bufs=4, space="PSUM") as ptpool:
                for c4 in range(CA // 512):
                    pt = ptpool.tile([128, 4, 128], dt.float32, tag="pa")
                    for c in range(4):
                        nc.tensor.transpose(
                            pt[:, c, :],
                            cacca[:, (c4 * 4 + c) * 128:(c4 * 4 + c + 1) * 128],
                            ident_sb[:],
                        )
                    ci = c4 * 4
                    nc.vector.tensor_reduce(
                        cminv[:, ci:ci + 4], pt[:], axis=ax_x, op=op_min)
                for c4 in range(CB // 512):
                    pt = ptpool.tile([128, 4, 128], dt.bfloat16, tag="pb")
                    for c in range(4):
                        nc.tensor.transpose(
                            pt[:, c, :],
                            caccb[:, (c4 * 4 + c) * 128:(c4 * 4 + c + 1) * 128],
                            identb[:],
                        )
                    ci = CA // 128 + c4 * 4
                    nc.vector.tensor_reduce(
                        cminv[:, ci:ci + 4], pt[:], axis=ax_x, op=op_min)

            nc.sync.dma_start(out[:, :NT], rminva[:])
            nc.sync.dma_start(out[:, NT:], cminv[:])
            nc.sync.dma_start(outb[:], rminvb[:])

    nc.compile()
    return nc


def _get_module():
    rep = int(os.environ.get("CHAMFER_REPEAT", "1"))
    key = ("nc", rep, CA, ROWG, NO2X)
    if key not in _CACHE:
        _CACHE[key] = build_module(repeat=rep)
    return _CACHE[key]


def kernel(X, Y):
    from concourse import bass_utils

    X = np.asarray(X)
    Y = np.asarray(Y)
    assert X.shape == (B, N, D) and Y.shape == (B, M, D)

    XAT, YAT = _augment(X, Y)
    ident = np.eye(128, dtype=np.float32)

    nc = _get_module()
    in_maps = [{"xat": XAT[b], "yat": YAT[b], "ident": ident} for b in range(B)]
    r = bass_utils.run_bass_kernel_spmd(nc, in_maps, core_ids=list(range(B)))
    _CACHE["last_results"] = r

    NT = N // 128
    outv = np.empty((B,), np.float32)
    for b in range(B):
        o = r.results[b]["out"].astype(np.float64)
        ob = r.results[b]["outb"].astype(np.float64)
        rmin = np.minimum(o[:, :NT], ob) if CB else o[:, :NT]
        cmin = o[:, NT:]
        outv[b] = np.float32(rmin.mean() + cmin.mean())
    return outv


# revision 3
# speedup vs baseline: 1.3875x; 1.0066x over previous
"""Chamfer distance kernel for Trainium2 (8 NeuronCores, bass/tile).

Problem: X [8, 8192, 3], Y [8, 8192, 3] fp32.
  out[b] = mean_n min_m ||x_n - y_m||^2 + mean_m min_n ||x_n - y_m||^2

Design (one batch per core; measured ~460us main loop vs 611us baseline):
  - W tiles [128, 8192] produced by the PE as a K=24 bf16 triple-split
    matmul (fp32-accurate distances). 16x 512-col matmuls per tile, with
    4-way tile_position row-group rotation (measured 3.5x faster than a
    single row group: 2.85us vs 9.96us per tile).
  - A fused min/min custom DVE op consumes W in a single pass:
      out = min(in0, in1)            (running column-min update)
      accum_out = min(s0, min(in1))  (row-min of the W stream only; the
        lowered accumulator is patched to tap Src1's carry lane so cacc
        values cannot contaminate row-mins)
    Two registered instances: CHAMFER_FUSED_ANT (REGULAR only, used on
    fp32 PSUM operands) and CHAMFER_F2X_ANT, which adds a hand-built
    2X_1PORT uop program (element 1 via SRC_*_HI lanes, pair-min at blk2
    feeding the accumulator; perf_max=1 on the instruction) and runs at
    2 elem/lane/cycle on all-bf16 operands (accum_out must be bf16 --
    fp32 accum corrupts the accumulator in 2x mode).
  - Columns split to balance DVE vs ACT: A-slice (CA=2048) consumed
    fp32 straight from PSUM at REGULAR rate; B-slice (6144) cast by ACT
    to bf16 wb, then consumed at 2x. The B consumption is
    software-pipelined one tile behind the casts so the in-order DVE
    never stalls on the current tile's ACT.
  - Row means: 2 accum entries per tile (A fp32 + B bf16) -> host mins.
  - Col means: PE-transpose of cacc chunks, DVE tensor_reduce -> cminv.
"""

import os
import sys

sys.path.insert(0, "/opt/trn_rl_repo")

import numpy as np

B, N, M, D = 8, 8192, 8192, 3
KROWS = 24
FILL = 30000.0

GRAIN = 2048
CA = int(os.environ.get("CHAMFER_CA", "2048"))
CB = M - CA
ROWG = int(os.environ.get("CHAMFER_RG", "4"))  # PE row-group rotation
NO2X = bool(int(os.environ.get("CHAMFER_NO2X", "0")))  # disable 2x (diagnostics)

_CACHE = {}


def _split3_bf16(v):
    import ml_dtypes

    bfdt = ml_dtypes.bfloat16
    v = v.astype(np.float64)
    s0 = v.astype(bfdt)
    r1 = v - s0.astype(np.float64)
    s1 = r1.astype(bfdt)
    r2 = r1 - s1.astype(np.float64)
    s2 = r2.astype(bfdt)
    return s0, s1, s2


def _augment(X, Y):
    """[B,24,N] lhsT rows and [B,24,M] rhs rows: sum_k XAT[k,n]*YAT[k,m] =
    |x_n|^2 + |y_m|^2 - 2 x_n.y_m to ~2^-26."""
    import ml_dtypes

    bfdt = ml_dtypes.bfloat16
    Xf = np.asarray(X, np.float64)
    Yf = np.asarray(Y, np.float64)
    X2 = (Xf * Xf).sum(-1)
    Y2 = (Yf * Yf).sum(-1)
    xs = _split3_bf16(np.moveaxis(Xf, -1, 1))
    ys = _split3_bf16(np.moveaxis(-2.0 * Yf, -1, 1))
    a = _split3_bf16(X2)
    b = _split3_bf16(Y2)

    nb, mb = X.shape[1], Y.shape[1]
    XAT = np.zeros((B, KROWS, nb), bfdt)
    YAT = np.zeros((B, KROWS, mb), bfdt)
    pairs = [(0, 0), (0, 1), (1, 0), (0, 2), (1, 1), (2, 0)]
    r = 0
    for d in range(D):
        for (i, j) in pairs:
            XAT[:, r, :] = xs[i][:, d, :]
            YAT[:, r, :] = ys[j][:, d, :]
            r += 1
    for i in range(3):
        XAT[:, r, :] = a[i]
        YAT[:, r, :] = np.ones((B, mb), bfdt)
        r += 1
    for i in range(3):
        XAT[:, r, :] = np.ones((B, nb), bfdt)
        YAT[:, r, :] = b[i]
        r += 1
    assert r == KROWS
    return XAT, YAT


def _register_fused(name, with_2x):
    """Fused min/min custom DVE op; optionally with a 2X_1PORT program.

    The 2x program requires all-bf16 operands including accum_out (fp32
    accum_out corrupts the accumulator on HW -- probed)."""
    from concourse import dve_ops
    from concourse.dve_spec import Spec, Src0, Src1, minn, lower, _has_src1
    from concourse.dve_uop import (
        DveOpSpec, AluInp, AluOp, UopConfig, InpSel, OutSel,
        OutPath, Trigger, DelayInp, ENABLE,
    )

    if name in _CACHE:
        return _CACHE[name]

    def _ref(in0, in1, s0, s1, imm2):
        b = np.minimum(in0.astype(np.float32), in1.astype(np.float32))
        seed = np.asarray(s0, np.float32).reshape(-1, 1)
        acc = np.minimum(
            in1.astype(np.float32).reshape(in1.shape[0], -1).min(-1, keepdims=True),
            seed,
        )
        return b, acc

    spec = Spec(body=minn(Src0, Src1), accum=minn, accum_init=dve_ops.C0,
                reference=_ref)
    op = dve_ops.DveOp(name, spec, subdim=False, uops_sha={}, perf_en={})
    row = max(dve_ops._SUB_OPCODE_FOR_NAME.values()) + 1
    assert row < 0x20
    dve_ops._SUB_OPCODE_FOR_NAME[op.name] = row
    dve_ops.OPS.append(op)
    dve_ops.CUSTOM_DVE_SPECS[op.name] = spec

    MIN, BYP = AluOp.MIN, AluOp.BYPASS
    PREV, CURR = AluInp.PREV_ALU_OUT, AluInp.CURR_ALU_OUT
    D_ = [AluInp.PREV_DELAY_0, AluInp.PREV_DELAY_1, AluInp.PREV_DELAY_2,
          AluInp.PREV_DELAY_3, AluInp.PREV_DELAY_4]

    def mk2x(seed):
        u = UopConfig()
        for j, sel in [(1, InpSel.SRC_0), (2, InpSel.SRC_1),
                       (3, InpSel.SRC_0_HI), (4, InpSel.SRC_1_HI),
                       (5, InpSel.CONST_0)]:
            u.enable_input(sel, j)
        dp = u.datapath_config
        dp[0].enable_alu(MIN, D_[0], D_[1])
        dp[0].pass_through_delay(1, 2, 3, 4)
        dp[1].enable_alu(MIN, D_[2], D_[3])
        dp[1].pass_through_delay(1, 2, 3, 4)
        dp[1].enable_delay_from_src(DelayInp.PREV_ALU_OUT, 0)  # o_lo
        dp[2].enable_alu(MIN, D_[1], D_[3])
        dp[2].pass_through_delay(0, 1, 3, 4)
        dp[2].enable_delay_from_src(DelayInp.PREV_ALU_OUT, 2)  # o_hi
        if seed:
            dp[3].enable_alu(BYP, D_[4], D_[4])
        else:
            dp[3].enable_alu(MIN, CURR, PREV)
        dp[3].pass_through_delay(0, 2, 4)
        dp[3].alu_out_a_enable = ENABLE
        for k in (4, 5, 6, 7):
            dp[k].enable_alu(BYP, PREV, PREV)
            dp[k].pass_through_delay(0, 2, 4)
            dp[k].alu_out_a_enable = ENABLE
        u.accum_enabled = ENABLE
        if seed:
            u.trigger = (Trigger.COUNT, Trigger.NONE, Trigger.NONE)
            u.repeat_count = 1
            u.next_uop = (1, 0, 0)
        else:
            u.enable_output(OutSel.DELAY_0, OutPath.WR0_LO)
            u.enable_output(OutSel.DELAY_2, OutPath.WR0_HI)
            u.require_inp0 = 1
            u.require_inp1 = 1
            u.trigger = (Trigger.SRC_TENSOR_DONE, Trigger.NONE, Trigger.NONE)
            u.next_uop = (0, 0, 0)
        return u

    for ver in ("v3",):
        uops = lower(spec, ver=ver)
        steady = uops[-1]
        patched = False
        for blk in steady.datapath_config:
            if blk.alu_src0 == AluInp.CURR_ALU_OUT and blk.alu_out_a_enable:
                assert blk.alu_src1 == AluInp.PREV_ALU_OUT
                blk.alu_src1 = AluInp.PREV_DELAY_1  # accum taps Src1, not body
                patched = True
                break
        assert patched
        if with_2x:
            s = DveOpSpec(name=op.name, opcode=row, uops=uops,
                          uops_2x=[mk2x(True), mk2x(False)], perf_max=1,
                          rd1_en=_has_src1(spec))
        else:
            s = DveOpSpec(name=op.name, opcode=row, uops=uops,
                          rd1_en=_has_src1(spec))
        s.validate(ver)
        op.uops_sha[ver] = s.sha(ver)
        dve_ops._COMPILE_CACHE[(op.name, ver)] = s
    _CACHE[name] = op
    return op


def build_module(repeat=1):
    import concourse.bacc as bacc
    import concourse.mybir as mybir
    import concourse.tile as tile
    from concourse._compat import get_trn_type

    dt = mybir.dt
    op_min = mybir.AluOpType.min
    ax_x = mybir.AxisListType.X
    fusedA = _register_fused("CHAMFER_FUSED_ANT", with_2x=False)
    fusedB = _register_fused("CHAMFER_F2X_ANT", with_2x=True)

    NT = N // 128
    NG = M // GRAIN
    ga = CA // GRAIN
    assert CA % GRAIN == 0

    nc = bacc.Bacc(get_trn_type() or "TRN2", target_bir_lowering=False,
                   debug=False)
    xat = nc.dram_tensor("xat", [KROWS, N], dt.bfloat16, kind="ExternalInput")
    yat = nc.dram_tensor("yat", [KROWS, M], dt.bfloat16, kind="ExternalInput")
    ident = nc.dram_tensor("ident", [128, 128], dt.float32, kind="ExternalInput")
    out = nc.dram_tensor("out", [128, NT + M // 128], dt.float32,
                         kind="ExternalOutput")
    outb = nc.dram_tensor("outb", [128, NT], dt.bfloat16,
                          kind="ExternalOutput")

    def fused_op(op, out_, in0, in1, accum_out, twox):
        bi = nc.vector._custom_dve(op, out=out_, in0=in0, in1=in1,
                                   s0=float(FILL), accum_out=accum_out)
        if twox and not NO2X:
            bi.ins.perf_max = 1
        return bi

    with tile.TileContext(nc) as tc:
        with (
            tc.tile_pool(name="const", bufs=1) as cpool,
            tc.tile_pool(name="acc", bufs=1) as apool,
            tc.tile_pool(name="res", bufs=1) as rpool,
        ):
            ident_sb = cpool.tile([128, 128], dt.float32)
            identb = cpool.tile([128, 128], dt.bfloat16)
            nc.sync.dma_start(ident_sb[:], ident[:])
            nc.scalar.copy(identb[:], ident_sb[:])
            if ROWG > 1:
                # replicate operands at partition offsets 0/32/64/96 so
                # matmuls rotate PE row groups (overlapped weight loads)
                xat_sb = cpool.tile([128, N], dt.bfloat16)
                yat_sb = cpool.tile([128, M], dt.bfloat16)
                for g in range(ROWG):
                    nc.sync.dma_start(xat_sb[32 * g:32 * g + KROWS, :], xat[:])
                    nc.sync.dma_start(yat_sb[32 * g:32 * g + KROWS, :], yat[:])
            else:
                xat_sb = cpool.tile([KROWS, N], dt.bfloat16)
                yat_sb = cpool.tile([KROWS, M], dt.bfloat16)
                nc.sync.dma_start(xat_sb[:], xat[:])
                nc.sync.dma_start(yat_sb[:], yat[:])

            cacca = apool.tile([128, CA], dt.float32, name="cacca") if CA else None
            caccb = apool.tile([128, CB], dt.bfloat16, name="caccb") if CB else None
            rminva = rpool.tile([128, NT], dt.float32)
            rminvb = rpool.tile([128, NT], dt.bfloat16)
            cminv = rpool.tile([128, M // 128], dt.float32)
            if CA:
                nc.vector.memset(cacca[:], FILL)
            if CB:
                nc.vector.memset(caccb[:], FILL)
            nc.vector.memset(rminva[:], FILL)
            nc.vector.memset(rminvb[:], FILL)

            with (
                tc.tile_pool(name="wb", bufs=2) as wbpool,
                tc.tile_pool(name="ps", bufs=2, space="PSUM") as pspool,
            ):
                # Software-pipeline the B-slice consumption by one tile:
                # fusedB(i-1) is emitted after tile i's casts are queued, so
                # the in-order DVE never stalls waiting for the current
                # tile's ACT casts.
                pend = [None]

                def flush_pend():
                    prev = pend[0]
                    if prev is not None:
                        fused_op(fusedB, caccb[:], caccb[:], prev[0],
                                 rminvb[:, prev[1]:prev[1] + 1], twox=True)
                        pend[0] = None

                def emit_tile(i):
                    wb = wbpool.tile([128, CB], dt.bfloat16,
                                     name="wb", tag="w") if CB else None
                    for g in range(NG):
                        ps = pspool.tile([128, GRAIN], dt.float32, tag="g")
                        for q in range(GRAIN // 512):
                            mo = g * GRAIN + q * 512
                            if ROWG > 1:
                                rg = (g * (GRAIN // 512) + q) % ROWG
                                nc.tensor.matmul(
                                    ps[:, q * 512:(q + 1) * 512],
                                    xat_sb[32 * rg:32 * rg + KROWS,
                                           i * 128:(i + 1) * 128],
                                    yat_sb[32 * rg:32 * rg + KROWS,
                                           mo:mo + 512],
                                    start=True, stop=True,
                                    tile_position=(32 * rg, 0),
                                )
                            else:
                                nc.tensor.matmul(
                                    ps[:, q * 512:(q + 1) * 512],
                                    xat_sb[:, i * 128:(i + 1) * 128],
                                    yat_sb[:, mo:mo + 512],
                                    start=True, stop=True,
                                )
                        if g < ga:
                            o = g * GRAIN
                            fused_op(fusedA, cacca[:, o:o + GRAIN],
                                     cacca[:, o:o + GRAIN], ps[:],
                                     rminva[:, i:i + 1], twox=False)
                        else:
                            o = (g - ga) * GRAIN
                            nc.scalar.copy(wb[:, o:o + GRAIN], ps[:])
                    if CB:
                        flush_pend()
                        pend[0] = (wb, i)

                if repeat > 1:
                    with tc.For_i(0, repeat, 1):
                        for i in range(NT):
                            emit_tile(i)
                        flush_pend()
                else:
                    for i in range(NT):
                        emit_tile(i)
                    flush_pend()

            # col finalization: PE transpose 128-chunks, DVE reduce
            with tc.tile_pool(name="pst", bufs=4, space="PSUM") as ptpool:
                for c4 in range(CA // 512):
                    pt = ptpool.tile([128, 4, 128], dt.float32, tag="pa")
                    for c in range(4):
                        nc.tensor.transpose(
                            pt[:, c, :],
                            cacca[:, (c4 * 4 + c) * 128:(c4 * 4 + c + 1) * 128],
                            ident_sb[:],
                        )
                    ci = c4 * 4
                    nc.vector.tensor_reduce(
                        cminv[:, ci:ci + 4], pt[:], axis=ax_x, op=op_min)
                for c4 in range(CB // 512):
                    pt = ptpool.tile([128, 4, 128], dt.bfloat16, tag="pb")
                    for c in range(4):
                        nc.tensor.transpose(
                            pt[:, c, :],
                            caccb[:, (c4 * 4 + c) * 128:(c4 * 4 + c + 1) * 128],
                            identb[:],
                        )
                    ci = CA // 128 + c4 * 4
                    nc.vector.tensor_reduce(
                        cminv[:, ci:ci + 4], pt[:], axis=ax_x, op=op_min)

            nc.sync.dma_start(out[:, :NT], rminva[:])
            nc.sync.dma_start(out[:, NT:], cminv[:])
            nc.sync.dma_start(outb[:], rminvb[:])

    nc.compile()
    return nc


def _get_module():
    rep = int(os.environ.get("CHAMFER_REPEAT", "1"))
    key = ("nc", rep, CA, ROWG, NO2X)
    if key not in _CACHE:
        _CACHE[key] = build_module(repeat=rep)
    return _CACHE[key]


def kernel(X, Y):
    from concourse import bass_utils

    X = np.asarray(X)
    Y = np.asarray(Y)
    assert X.shape == (B, N, D) and Y.shape == (B, M, D)

    XAT, YAT = _augment(X, Y)
    ident = np.eye(128, dtype=np.float32)

    nc = _get_module()
    in_maps = [{"xat": XAT[b], "yat": YAT[b], "ident": ident} for b in range(B)]
    r = bass_utils.run_bass_kernel_spmd(nc, in_maps, core_ids=list(range(B)))
    _CACHE["last_results"] = r

    NT = N // 128
    outv = np.empty((B,), np.float32)
    for b in range(B):
        o = r.results[b]["out"].astype(np.float64)
        ob = r.results[b]["outb"].astype(np.float64)
        rmin = np.minimum(o[:, :NT], ob) if CB else o[:, :NT]
        cmin = o[:, NT:]
        outv[b] = np.float32(rmin.mean() + cmin.mean())
    return outv
